# revision 2
# baseline (speedup 1.0000x reference)
"""Causal multi-head attention (B=4, T=2048, C=1024, H=16, HD=64) on 8 trn2 cores.

Sharding: core i -> batch b = i//2, head-half hh = i%2 (8 heads = 512 dims).
Host sums the two head-half partials per batch (bf16 partials -> f32).

v2 dataflow (fp8/bf16 mixed precision, validated to rel err ~3.5e-3):
  - Projections q/k/v via fp8 DoubleRow matmuls, 3-term residual decomposition
    (xh*Wh + xl*Wh + xh*Wl) with power-of-2 scale matching: x8h=fp8(8x),
    x8l=fp8(8(x-xh)), w8=fp8(512W), w8l=fp8(512(W-wh)). PSUM holds 4096*q.
  - v computed directly in [t, d] layout (x-slice stationary), quantized to
    fp8 hi (16v, via ACT copy scale 1/256) + lo residual (gpsimd
    scalar_tensor_tensor) into ones/zeros-augmented vA/vB tiles.
  - q/k copied to bf16 SBUF (4096q); S^T tiles via bf16 matmuls (K=64),
    causally trimmed widths near the diagonal.
  - exp on ACT with scale 2^-27, bias -7+ln16, writing fp8 e directly;
    diagonal-block triangle masks multiplied on gpsimd (Pool) in fp8.
  - PV via fp8 DoubleRow (2 tk-tiles per pass), hi + lo terms; ones-aug
    gives the softmax denominator for free.
  - normalize: DVE reciprocal + multiply -> bf16 att (= 16*att) resident in
    SBUF (no DRAM spill).
  - Wo phase: bf16 matmuls from resident att; wo16 = bf16(Wo.T/16) preloaded;
    y stored bf16, host upcasts and sums partials.
"""

import os
import sys
from contextlib import ExitStack

import numpy as np

try:
    from concourse import bass, tile, mybir
except ImportError:  # pragma: no cover
    sys.path.insert(0, "/opt/trn_rl_repo")
    from concourse import bass, tile, mybir

import ml_dtypes

from concourse.bass2jax import _bass_exec_p, install_neuronx_cc_hook

F32 = mybir.dt.float32
BF16 = mybir.dt.bfloat16
F8 = mybir.dt.float8e4
AF = mybir.ActivationFunctionType
ALU = mybir.AluOpType
DR = mybir.MatmulPerfMode.DoubleRow
NPF8 = ml_dtypes.float8_e4m3
NPBF16 = ml_dtypes.bfloat16

B, T, C = 4, 2048, 1024
H, HD = 16, 64
NCORES = 8
HH = 512          # head-dims per core (8 heads)
NPAIR = 4         # head-pairs per core (128 dims each)
NCC = C // 128    # 8 contraction chunks
NG = NCC // 2     # 4 DoubleRow chunk-pairs
NTT = T // 128    # 16 t-tiles
NTQ = T // 512    # 4 query chunks

EXP_SCALE = 1.0 / float(2 ** 27)          # undo 4096^2 * 8
EXP_BIAS = -7.0 + float(np.log(16.0))     # e = 16*exp(s-7), max ~123 < 240

_PROGRAM = None
last_run_info = {}
DEBUG_DUMPS = False  # add qT/vA/att DRAM dumps for stage-wise debugging


def _build_program():
    nc = bass.Bass("TRN2", target_bir_lowering=False, debug=False)

    x8h_d = nc.declare_dram_parameter("x8h", [C, T], F8, isOutput=False)
    x8l_d = nc.declare_dram_parameter("x8l", [C, T], F8, isOutput=False)
    w_d = {}
    for nm in ("wq", "wk", "wv", "wql", "wkl", "wvl"):
        w_d[nm] = nc.declare_dram_parameter(nm, [C, HH], F8, isOutput=False)
    wo_d = nc.declare_dram_parameter("wo", [HH, C], BF16, isOutput=False)
    y_d = nc.declare_dram_parameter("y", [T, C], BF16, isOutput=True)

    tri = np.tril(np.ones((128, 128), np.float32)).T  # tri[p, c] = 1 if p <= c
    tri_c = nc.inline_tensor(tri.astype(NPF8), "tric")
    trib_c = nc.inline_tensor(tri.astype(NPBF16), "tribc")

    x8h = x8h_d.ap()
    x8l = x8l_d.ap()
    wo = wo_d.ap()
    y = y_d.ap()

    with tile.TileContext(nc) as tc, ExitStack() as ctx:
        cst = ctx.enter_context(tc.tile_pool(name="cst", bufs=1))
        qkp = ctx.enter_context(tc.tile_pool(name="qkp", bufs=2))
        esp = ctx.enter_context(tc.tile_pool(name="esp", bufs=3))
        stp = ctx.enter_context(tc.tile_pool(name="stp", bufs=2))
        ybp = ctx.enter_context(tc.tile_pool(name="ybp", bufs=4))
        pp = ctx.enter_context(tc.tile_pool(name="pp", bufs=2, space="PSUM"))
        psp = ctx.enter_context(tc.tile_pool(name="psp", bufs=3, space="PSUM"))

        # ---- constants / persistent tiles
        tri8 = cst.tile([128, 128], F8, tag="tri8")
        nc.sync.dma_start(tri8[:], tri_c.ap())
        trib = cst.tile([128, 128], BF16, tag="trib")
        nc.sync.dma_start(trib[:], trib_c.ap())
        ebias = cst.tile([128, 1], F32, tag="ebias")
        nc.gpsimd.memset(ebias[:], EXP_BIAS)

        vA = cst.tile([128, NTT, NPAIR, 128], F8, tag="vA")
        vB = cst.tile([128, NTT, NPAIR, 128], F8, tag="vB")
        vAl = cst.tile([128, NTT, NPAIR, 128], F8, tag="vAl")
        vBl = cst.tile([128, NTT, NPAIR, 128], F8, tag="vBl")
        nc.gpsimd.memset(vA[:, :, :, 64:128], 1.0)
        nc.gpsimd.memset(vB[:, :, :, 64:128], 1.0)
        nc.gpsimd.memset(vAl[:, :, :, 64:128], 0.0)
        nc.gpsimd.memset(vBl[:, :, :, 64:128], 0.0)
        # bf16 v copies of the first 4 tk-tiles: the jq=0 chunk runs in bf16
        # (fp8 e noise is undamped on short attention rows)
        vAb = cst.tile([128, 4, NPAIR, 128], BF16, tag="vAb")
        vBb = cst.tile([128, 4, NPAIR, 128], BF16, tag="vBb")
        nc.gpsimd.memset(vAb[:, :, :, 64:128], 1.0)
        nc.gpsimd.memset(vBb[:, :, :, 64:128], 1.0)

        att = cst.tile([128, NPAIR, T], BF16, tag="att")

        # ---- weight + x loads (v weights/x first: v-phase starts earliest)
        w_sb = {}

        def load_w(nm):
            wt = cst.tile([128, NCC, HH], F8, tag=nm, name=nm)
            for cc in range(NCC):
                nc.sync.dma_start(wt[:, cc, :], w_d[nm].ap()[cc * 128:(cc + 1) * 128, :])
            w_sb[nm] = wt

        load_w("wv")
        load_w("wvl")
        xh_t = cst.tile([128, NCC, T], F8, tag="xh")
        xl_t = cst.tile([128, NCC, T], F8, tag="xl")
        for cc in range(NCC):
            nc.sync.dma_start(xh_t[:, cc, :], x8h[cc * 128:(cc + 1) * 128, :])
        for cc in range(NCC):
            nc.sync.dma_start(xl_t[:, cc, :], x8l[cc * 128:(cc + 1) * 128, :])
        load_w("wq")
        load_w("wql")
        load_w("wk")
        load_w("wkl")
        wo_t = cst.tile([128, NPAIR, C], BF16, tag="wo")
        for p_ in range(NPAIR):
            nc.sync.dma_start(wo_t[:, p_, :], wo[p_ * 128:(p_ + 1) * 128, :])

        # ---- emission helpers (software pipelining across phases)
        def emit_v(tts):
            """v in [t, d] layout for all pairs; quantize hi/lo into vA/vB."""
            for tt in tts:
                vp = pp.tile([128, NPAIR, 128], F32, tag="pp", name="vp")
                for g in range(NG):
                    for ti, (xs, ws) in enumerate(
                        ((xh_t, "wv"), (xh_t, "wvl"), (xl_t, "wv"))
                    ):
                        nc.tensor.matmul(
                            vp.rearrange("p a b -> p (a b)"),
                            xs[:, 2 * g:2 * g + 2, tt * 128:(tt + 1) * 128],
                            w_sb[ws][:, 2 * g:2 * g + 2, :],
                            start=(g == 0 and ti == 0),
                            stop=(g == NG - 1 and ti == 2),
                            perf_mode=DR,
                        )
                # hi: fp8(16v) via DVE scaled copy (psum holds 4096v)
                nc.vector.tensor_scalar_mul(vA[:, tt, :, 0:64], vp[:, :, 0:64],
                                            1.0 / 256.0)
                nc.vector.tensor_scalar_mul(vB[:, tt, :, 0:64],
                                            vp[:, :, 64:128], 1.0 / 256.0)
                # lo: fp8(16v - hi) on DVE (gpsimd cannot access PSUM)
                nc.vector.scalar_tensor_tensor(
                    vAl[:, tt, :, 0:64], vp[:, :, 0:64], 1.0 / 256.0,
                    vA[:, tt, :, 0:64], ALU.mult, ALU.subtract)
                nc.vector.scalar_tensor_tensor(
                    vBl[:, tt, :, 0:64], vp[:, :, 64:128], 1.0 / 256.0,
                    vB[:, tt, :, 0:64], ALU.mult, ALU.subtract)
                if tt < 4:
                    nc.vector.tensor_scalar_mul(vAb[:, tt, :, 0:64],
                                                vp[:, :, 0:64], 1.0 / 256.0)
                    nc.vector.tensor_scalar_mul(vBb[:, tt, :, 0:64],
                                                vp[:, :, 64:128], 1.0 / 256.0)

        qk_tiles = {}

        def emit_proj_piece(p, piece):
            """q/k projection for pair p, quarter `piece` (mat x tg-group)."""
            if piece == 0:
                qk_tiles[p] = (
                    qkp.tile([128, T], BF16, tag="qT", name="qT"),
                    qkp.tile([128, T], BF16, tag="kT", name="kT"),
                )
            mi, tg = divmod(piece, 2)
            nm = ("wq", "wk")[mi]
            dst = qk_tiles[p][mi]
            lo = nm + "l"
            accs = [pp.tile([128, 512], F32, tag="pp", name=f"acc{i}")
                    for i in range(2)]
            for g in range(NG):
                for ti, (ws, xs) in enumerate(
                    ((nm, xh_t), (lo, xh_t), (nm, xl_t))
                ):
                    stat = w_sb[ws][:, 2 * g:2 * g + 2, p * 128:(p + 1) * 128]
                    for i in range(2):
                        t4 = 2 * tg + i
                        nc.tensor.matmul(
                            accs[i],
                            stat,
                            xs[:, 2 * g:2 * g + 2, t4 * 512:(t4 + 1) * 512],
                            start=(g == 0 and ti == 0),
                            stop=(g == NG - 1 and ti == 2),
                            perf_mode=DR,
                        )
            for i in range(2):
                t4 = 2 * tg + i
                nc.vector.tensor_copy(dst[:, t4 * 512:(t4 + 1) * 512], accs[i])

        def emit_attn_chunk(p, jq, h):
            qT, kT = qk_tiles[p]
            r0 = h * 64
            vh_, vl_ = (vA, vAl) if h == 0 else (vB, vBl)
            pa = pp.tile([128, 512], F32, tag="pp", name="pa")
            first_pv = [True]

            def pv(e_t, jt0, last=False):
                for vi, v_t in enumerate((vh_, vl_)):
                    nc.tensor.matmul(
                        pa[:],
                        v_t[:, jt0:jt0 + 2, p, :],
                        e_t[:],
                        start=first_pv[0],
                        stop=(last and vi == 1),
                        perf_mode=DR,
                    )
                    first_pv[0] = False

            # full tk-tile pairs
            for m in range(2 * jq):
                jt0 = 2 * m
                s = psp.tile([128, 1024], F32, tag="s")
                for j in range(2):
                    nc.tensor.matmul(
                        s[:, j * 512:(j + 1) * 512],
                        kT[r0:r0 + 64, (jt0 + j) * 128:(jt0 + j + 1) * 128],
                        qT[r0:r0 + 64, jq * 512:(jq + 1) * 512],
                        start=True, stop=True,
                    )
                e = esp.tile([128, 2, 512], F8, tag="e")
                nc.scalar.activation(
                    e.rearrange("p a b -> p (a b)"), s[:],
                    AF.Exp, scale=EXP_SCALE, bias=ebias[:, 0:1])
                pv(e, jt0)

            # diagonal region: tiles 4jq..4jq+3, trimmed widths.
            # jq=0 runs in bf16 (short rows do not damp fp8 e noise).
            bf = jq == 0
            for g2 in range(2):
                s = psp.tile([128, 1024], F32, tag="s")
                if bf:
                    e = esp.tile([128, 2, 512], BF16, tag="ebf")
                else:
                    e = esp.tile([128, 2, 512], F8, tag="e")
                for j in range(2):
                    d0 = 2 * g2 + j
                    off = 128 * d0
                    jt = 4 * jq + d0
                    nc.tensor.matmul(
                        s[:, j * 512 + off:(j + 1) * 512],
                        kT[r0:r0 + 64, jt * 128:(jt + 1) * 128],
                        qT[r0:r0 + 64, jq * 512 + off:(jq + 1) * 512],
                        start=True, stop=True,
                    )
                    if off:
                        nc.gpsimd.memset(e[:, j, 0:off], 0.0)
                    nc.scalar.activation(
                        e[:, j, off:512], s[:, j * 512 + off:(j + 1) * 512],
                        AF.Exp, scale=EXP_SCALE, bias=ebias[:, 0:1])
                    nc.gpsimd.tensor_tensor(
                        e[:, j, off:off + 128], e[:, j, off:off + 128],
                        trib[:] if bf else tri8[:], ALU.mult)
                if bf:
                    vb_ = vAb if h == 0 else vBb
                    for j in range(2):
                        nc.tensor.matmul(
                            pa[:],
                            vb_[:, 2 * g2 + j, p, :],
                            e[:, j, :],
                            start=first_pv[0],
                            stop=(g2 == 1 and j == 1),
                        )
                        first_pv[0] = False
                else:
                    pv(e, 4 * jq + 2 * g2, last=(g2 == 1))

            # normalize: att[h, p, chunk] = pa[0:64]/pa[64:128] (bf16)
            rc = stp.tile([64, 512], F32, tag="rc")
            nc.vector.reciprocal(rc[:], pa[64:128, :])
            nc.vector.tensor_tensor(
                att[r0:r0 + 64, p, jq * 512:(jq + 1) * 512],
                pa[0:64, :], rc[:], ALU.mult)

        def emit_wo(tts):
            """y[tt, :] = sum_p att[:, p, tt-block].T @ wo[p]"""
            for tt in tts:
                for ch in range(2):
                    yacc = pp.tile([128, 512], F32, tag="pp", name="yacc")
                    for p in range(NPAIR):
                        nc.tensor.matmul(
                            yacc[:],
                            att[:, p, tt * 128:(tt + 1) * 128],
                            wo_t[:, p, ch * 512:(ch + 1) * 512],
                            start=(p == 0), stop=(p == NPAIR - 1),
                        )
                    yb = ybp.tile([128, 512], BF16, tag="yb")
                    nc.vector.tensor_copy(yb[:], yacc[:])
                    nc.sync.dma_start(
                        y[tt * 128:(tt + 1) * 128,
                          ch * 512:(ch + 1) * 512], yb[:])

        if DEBUG_DUMPS:
            dumps = {
                "attdump": (att, [128, NPAIR * T], BF16),
                "qdump": None,  # filled after proj
                "vdump": (vA, [128, NTT * NPAIR * 128], F8),
                "vldump": (vAl, [128, NTT * NPAIR * 128], F8),
            }
            dump_d = {
                k: nc.declare_dram_parameter(k, v[1], v[2], isOutput=True)
                for k, v in dumps.items() if v
            }
            qdump_d = nc.declare_dram_parameter("qdump", [128, 2 * T], BF16,
                                                isOutput=True)

        # ---- schedule: pipeline v-groups / next-pair proj / Wo into the
        # ACT-bound attention chunks. v first: its DMAs arrive first.
        emit_v([0, 1, 2, 3])
        for piece in range(4):
            emit_proj_piece(0, piece)
        if DEBUG_DUMPS:
            nc.sync.dma_start(qdump_d.ap()[:, 0:T], qk_tiles[0][0][:])
            nc.sync.dma_start(qdump_d.ap()[:, T:2 * T], qk_tiles[0][1][:])
        for p in range(NPAIR):
            ci = 0  # chunk slot index within this pair
            for jq in range(NTQ):
                for h in range(2):
                    emit_attn_chunk(p, jq, h)
                    if p == 0 and ci < 3:
                        emit_v(range(4 * ci + 4, 4 * ci + 8))
                    if p < 3 and 3 <= ci < 7:
                        emit_proj_piece(p + 1, ci - 3)
                    if p == 3 and h == 1:
                        emit_wo(range(4 * jq, 4 * jq + 4))
                    ci += 1

        if DEBUG_DUMPS:
            flat = {"attdump": att.rearrange("p a b -> p (a b)"),
                    "vdump": vA.rearrange("p a b c -> p (a b c)"),
                    "vldump": vAl.rearrange("p a b c -> p (a b c)")}
            for k, ap in flat.items():
                nc.sync.dma_start(dump_d[k].ap(), ap)

    _split_matmul_waits(nc)
    return nc


def _split_matmul_waits(nc):
    """walrus's fused-LDW matmul lowering can't carry multiple sync waits
    (S3_LW setupSyncWait assert). Move excess waits onto a preceding
    same-engine NoOp, which lowers with full sync support."""
    f = nc.m.functions[0]
    k = 0
    for bb in f.blocks:
        insts = bb.instructions
        out = []
        for i in insts:
            waits = list(i.sync_info.on_wait) if i.sync_info is not None else []
            keep = 0 if type(i).__name__ == "InstMatmult" else 1
            if len(waits) > keep:
                moved, kept = waits[: len(waits) - keep], waits[len(waits) - keep:]
                for w in moved:
                    n = mybir.InstNoOp(name=f"I-mmwait{k}")
                    k += 1
                    n.engine = i.engine
                    n.sync_info = mybir.SyncInfo(on_wait=[w], on_update=[])
                    nc.register_instruction(n)
                    out.append(n)
                i.sync_info = mybir.SyncInfo(
                    on_wait=kept, on_update=list(i.sync_info.on_update)
                )
            out.append(i)
        if k:
            bb.instructions = out


def _get_program():
    global _PROGRAM
    if _PROGRAM is None:
        _PROGRAM = _build_program()
    return _PROGRAM


_RUNNER = None


def _get_runner():
    """Compile the SPMD program into a cached sharded jit callable."""
    global _RUNNER
    if _RUNNER is not None:
        return _RUNNER
    import jax
    from jax.experimental.shard_map import shard_map
    from jax.sharding import Mesh, PartitionSpec

    nc = _get_program()
    install_neuronx_cc_hook()

    partition_name = (
        nc.partition_id_tensor.name if nc.partition_id_tensor else None
    )
    in_names, out_names, out_avals = [], [], []
    for alloc in nc.m.functions[0].allocations:
        if not isinstance(alloc, mybir.MemoryLocationSet):
            continue
        name = alloc.memorylocations[0].name
        if alloc.kind == "ExternalInput":
            if name != partition_name:
                in_names.append(name)
        elif alloc.kind == "ExternalOutput":
            out_names.append(name)
            out_avals.append(
                jax.core.ShapedArray(tuple(alloc.tensor_shape), mybir.dt.np(alloc.dtype))
            )
    n_params = len(in_names)
    zero_outs = [np.zeros(a.shape, a.dtype) for a in out_avals]
    all_in_names = list(in_names) + list(out_names)
    if partition_name is not None:
        all_in_names.append(partition_name)
    all_in_names = tuple(all_in_names)

    def _body(*args):
        operands = list(args)
        if partition_name is not None:
            from concourse.bass2jax import partition_id_tensor

            operands.append(partition_id_tensor())
        outs = _bass_exec_p.bind(
            *operands,
            out_avals=tuple(out_avals),
            in_names=all_in_names,
            out_names=tuple(out_names),
            lowering_input_output_aliases=(),
            sim_require_finite=True,
            sim_require_nnan=True,
            nc=nc,
        )
        return tuple(outs)

    devices = jax.devices()[:NCORES]
    assert len(devices) == NCORES, devices
    mesh = Mesh(np.asarray(devices), ("core",))
    n_all = n_params + len(out_names)
    sharded = jax.jit(
        shard_map(
            _body,
            mesh=mesh,
            in_specs=(PartitionSpec("core"),) * n_all,
            out_specs=(PartitionSpec("core"),) * len(out_names),
            check_rep=False,
        ),
        keep_unused=True,
    )
    _RUNNER = dict(
        sharded=sharded,
        in_names=in_names,
        out_names=out_names,
        out_avals=out_avals,
        zero_outs=zero_outs,
        mesh=mesh,
    )
    return _RUNNER


def _run(in_maps):
    r = _get_runner()
    concat_in = [
        np.concatenate([np.asarray(m[name]) for m in in_maps], axis=0)
        for name in r["in_names"]
    ]
    concat_zeros = [
        np.zeros((NCORES * z.shape[0], *z.shape[1:]), z.dtype) for z in r["zero_outs"]
    ]
    out_arrs = r["sharded"](*concat_in, *concat_zeros)
    return [
        {
            name: np.asarray(out_arrs[i]).reshape(NCORES, *r["out_avals"][i].shape)[c]
            for i, name in enumerate(r["out_names"])
        }
        for c in range(NCORES)
    ]


def timed_run(in_maps, iters=10):
    """Execute with inputs pre-staged on device; return per-iteration seconds."""
    import time
    import jax

    r = _get_runner()
    concat_in = [
        np.concatenate([np.asarray(m[name]) for m in in_maps], axis=0)
        for name in r["in_names"]
    ]
    concat_zeros = [
        np.zeros((NCORES * z.shape[0], *z.shape[1:]), z.dtype) for z in r["zero_outs"]
    ]
    from jax.sharding import NamedSharding, PartitionSpec

    sh = NamedSharding(r["mesh"], PartitionSpec("core"))
    args = [jax.device_put(a, sh) for a in concat_in + concat_zeros]
    out = r["sharded"](*args)  # warmup + compile
    jax.block_until_ready(out)
    times = []
    for _ in range(iters):
        t0 = time.perf_counter()
        out = r["sharded"](*args)
        jax.block_until_ready(out)
        times.append(time.perf_counter() - t0)
    return times


def _q8(a, scale):
    a = np.asarray(a, np.float32) * scale
    np.clip(a, -240.0, 240.0, out=a)
    return a.astype(NPF8)


def make_in_maps(x, Wq, Wk, Wv, Wo):
    x = np.asarray(x, dtype=np.float32)
    Wq = np.asarray(Wq, dtype=np.float32)
    Wk = np.asarray(Wk, dtype=np.float32)
    Wv = np.asarray(Wv, dtype=np.float32)
    Wo = np.asarray(Wo, dtype=np.float32)

    xhs, xls = [], []
    for b in range(B):
        xt = np.ascontiguousarray(x[b].T)
        xh = _q8(xt, 8.0)
        xl = _q8(xt - xh.astype(np.float32) / 8.0, 8.0)
        xhs.append(xh)
        xls.append(xl)

    def whl(W, sl):
        wt = np.ascontiguousarray(W[sl, :].T)  # [C, HH]
        wh = _q8(wt, 512.0)
        wl = _q8(wt - wh.astype(np.float32) / 512.0, 512.0)
        return wh, wl

    in_maps = []
    for core in range(NCORES):
        b, hh = core // 2, core % 2
        sl = slice(hh * HH, (hh + 1) * HH)
        wq, wql = whl(Wq, sl)
        wk, wkl = whl(Wk, sl)
        wv, wvl = whl(Wv, sl)
        wo16 = (np.ascontiguousarray(Wo[:, sl].T) / 16.0).astype(NPBF16)
        in_maps.append({
            "x8h": xhs[b], "x8l": xls[b],
            "wq": wq, "wql": wql, "wk": wk, "wkl": wkl,
            "wv": wv, "wvl": wvl, "wo": wo16,
        })
    return in_maps


def kernel(x, Wq, Wk, Wv, Wo):
    in_maps = make_in_maps(x, Wq, Wk, Wv, Wo)
    results = _run(in_maps)
    out = np.empty((B, T, C), dtype=np.float32)
    for b in range(B):
        out[b] = (results[2 * b]["y"].astype(np.float32)
                  + results[2 * b + 1]["y"].astype(np.float32))
    return out
